# revision 3
# baseline (speedup 1.0000x reference)
"""Trainium2 Bass kernel for nn_AutoEncoder (6-layer GCN autoencoder).

Strategy (8 NeuronCores, SPMD):
  - Destination nodes sharded across cores (6250/core, padded to 6272).
  - Node features kept pre-scaled by deg^-1/2 ("hs") and replicated on every
    core in a padded [8*6272, F] layout (per-layer AllGather).
  - Per layer: dma_gather of hs[src] for this core's edges (edge list sorted
    by local dst, split by int16-index halves), segment-sum via one-hot
    matmuls accumulated in PSUM (128-dst windows), scale by deg^-1/2[dst],
    W matmul, BatchNorm with cross-core AllReduce of (sum, sumsq), ReLU,
    rescale by deg^-1/2, PE transpose back to node-major, AllGather.
  - Self-loops and psum-coverage for pad dst rows are injected as extra
    edges on the host; the GCN bias b is skipped (training-mode BatchNorm
    makes any per-feature constant shift a no-op).
"""

import sys

sys.path.insert(0, "/opt/trn_rl_repo")

import numpy as np

N = 50000
E = 800000
F_IN = 128
EPS = 1e-5
NC = 8
SH = 6250  # real dst nodes per core
SHP = 6272  # padded (49 * 128)
NP = NC * SHP  # 50176 rows in the padded replicated node table
HALF = NP // 2  # 25088 (< int16 max) rows per gather table half
WIN = 128  # dst window = psum column band
NWIN = SHP // WIN  # 49
NSB = 13  # psum superblocks: 12 x 512 + 1 x 128
CHUNK = 64  # gather chunk size in K-tiles
DIMS = [(128, 128), (128, 128), (128, 64), (64, 128), (128, 128), (128, 128)]
RELU = [True, True, False, True, True, False]

TRACE = False  # set by test.py for profiling runs
TRACE_KW = {}
LAST_RESULT = None  # BassKernelResults of the last run (for test.py)


def _prep_edges(src_remap, dstl):
    """Per-core edge preprocessing: sort by dst, window/half tiling.

    Returns dict with per-(window, half) edge lists:
      ed[w][h] = (gidx_local int32 array, dstl_rel int32 array)
    """
    order = np.argsort(dstl, kind="stable")
    dstl = dstl[order]
    srcr = src_remap[order]
    half = (srcr >= HALF).astype(np.int64)
    w = dstl // WIN
    rel = dstl - w * WIN
    ed = [[None, None] for _ in range(NWIN)]
    for wi in range(NWIN):
        m = w == wi
        for h in (0, 1):
            mh = m & (half == h)
            ed[wi][h] = (srcr[mh] - h * HALF, rel[mh])
    return ed


def _build_core_tables(ed, tiles):
    """Pack per-core edge lists into padded tile streams.

    tiles[w][h] = uniform (cross-core) tile count for that window/half.
    Returns per half: gidx [T*128] int16, dstl_rel [T*128] float32,
    and the per-window start tile t0[w][h].
    """
    out = []
    for h in (0, 1):
        T = sum(tiles[w][h] for w in range(NWIN))
        gidx = np.zeros(T * 128, np.int16)
        drel = np.full(T * 128, -1.0, np.float32)
        t = 0
        for w in range(NWIN):
            g, r = ed[w][h]
            nt = tiles[w][h]
            assert len(g) <= nt * 128
            # lay edges into tiles: edge k of this window -> tile t + k//128,
            # partition k%128
            base = t * 128
            gidx[base : base + len(g)] = g.astype(np.int16)
            drel[base : base + len(g)] = r.astype(np.float32)
            t += nt
        out.append((gidx, drel))
    return out


def _wrap_idx(gidx, chunks):
    """int16 gather indices -> [128, total/16] wrapped layout.

    chunks: list of (tile_start, ntiles). Each chunk's indices are wrapped
    independently: within a chunk, index i lives at [i%16, chunk_col0+i//16],
    then the 16 rows are tiled 8x across 128 partitions.
    """
    total_cols = len(gidx) // 16
    arr = np.zeros((16, total_cols), np.int16)
    col = 0
    for t0, nt in chunks:
        cidx = gidx[t0 * 128 : (t0 + nt) * 128]
        ncol = len(cidx) // 16
        arr[:, col : col + ncol] = cidx.reshape(ncol, 16).T
        col += ncol
    assert col == total_cols
    return np.tile(arr, (8, 1)).copy()


def _chunks_of(T):
    out = []
    t = 0
    while t < T:
        nt = min(CHUNK, T - t)
        out.append((t, nt))
        t += nt
    return out


def _build_program(tiles, t0s, TA, TB):
    from concourse import bacc, mybir, tile

    FP32 = mybir.dt.float32
    I16 = mybir.dt.int16
    AX = mybir.AxisListType.X
    OP = mybir.AluOpType
    ACTF = mybir.ActivationFunctionType

    nc = bacc.Bacc(None, num_devices=NC, target_bir_lowering=False, debug=False)

    # ---- parameters ----
    hs0_d = nc.declare_dram_parameter("hs0", [NP, F_IN], FP32, isOutput=False)
    idx_d = [
        nc.declare_dram_parameter("idxA", [128, TA * 8], I16, isOutput=False),
        nc.declare_dram_parameter("idxB", [128, TB * 8], I16, isOutput=False),
    ]
    dstl_d = [
        nc.declare_dram_parameter("dstlA", [128, TA], FP32, isOutput=False),
        nc.declare_dram_parameter("dstlB", [128, TB], FP32, isOutput=False),
    ]
    dinv_d = nc.declare_dram_parameter("dinvT", [128, SHP], FP32, isOutput=False)
    iota_d = nc.declare_dram_parameter("iota", [128, 128], FP32, isOutput=False)
    ident_d = nc.declare_dram_parameter("ident", [128, 128], FP32, isOutput=False)
    W_d = [
        nc.declare_dram_parameter(f"W{j}", list(DIMS[j]), FP32, isOutput=False)
        for j in range(6)
    ]
    gb_d = [
        nc.declare_dram_parameter(f"gb{j}", [128, 2], FP32, isOutput=False)
        for j in range(6)
    ]
    out_d = nc.declare_dram_parameter("out", [128, SHP], FP32, isOutput=True)

    # ---- internal DRAM: collective bounce buffers ----
    ag_in = [nc.dram_tensor(f"ag_in{j}", [SHP, DIMS[j][1]], FP32) for j in range(5)]
    ag_out = [
        nc.dram_tensor(f"ag_out{j}", [NP, DIMS[j][1]], FP32, addr_space="Shared")
        for j in range(5)
    ]
    ar_in = [nc.dram_tensor(f"ar_in{j}", [128, 2], FP32) for j in range(6)]
    ar_out = [
        nc.dram_tensor(f"ar_out{j}", [128, 2], FP32, addr_space="Shared")
        for j in range(6)
    ]

    tbls = [hs0_d] + ag_out  # gather table per layer

    chunksA = _chunks_of(TA)
    chunksB = _chunks_of(TB)
    chunks = [chunksA, chunksB]
    # chunk column offsets in the wrapped idx array
    idx_col0 = [[], []]
    for h in (0, 1):
        c = 0
        for _, nt in chunks[h]:
            idx_col0[h].append(c)
            c += nt * 8

    with tile.TileContext(nc) as tc:
        with (
            tc.tile_pool(name="res", bufs=1) as res,
            tc.tile_pool(name="msg", bufs=2) as msgp,
            tc.tile_pool(name="sp", bufs=4) as sp,
            tc.tile_pool(name="small", bufs=2) as small,
            tc.tile_pool(name="big", bufs=1) as big,
            tc.tile_pool(name="agg_ps", bufs=2, space="PSUM") as aggp,
            tc.tile_pool(name="y_ps", bufs=2, space="PSUM") as yp,
            tc.tile_pool(name="tr_ps", bufs=2, space="PSUM") as trp,
        ):
            # ---- resident loads ----
            idx_t = [res.tile([128, TA * 8], I16, tag="idxA", name="idxA")]
            nc.sync.dma_start(idx_t[0][:], idx_d[0][:])
            idx_t.append(res.tile([128, TB * 8], I16, tag="idxB", name="idxB"))
            nc.sync.dma_start(idx_t[1][:], idx_d[1][:])
            dstl_t = [res.tile([128, TA], FP32, tag="dstlA", name="dstlA")]
            nc.sync.dma_start(dstl_t[0][:], dstl_d[0][:])
            dstl_t.append(res.tile([128, TB], FP32, tag="dstlB", name="dstlB"))
            nc.sync.dma_start(dstl_t[1][:], dstl_d[1][:])
            iota_t = res.tile([128, 128], FP32, tag="iota", name="iota")
            nc.sync.dma_start(iota_t[:], iota_d[:])
            ident_t = res.tile([128, 128], FP32, tag="ident", name="ident")
            nc.sync.dma_start(ident_t[:], ident_d[:])
            W_t = []
            for j in range(6):
                wt = res.tile(list(DIMS[j]), FP32, tag=f"W{j}")
                nc.sync.dma_start(wt[:], W_d[j][:])
                W_t.append(wt)
            gb_t = []
            for j in range(6):
                gt = res.tile([128, 2], FP32, tag=f"gb{j}")
                nc.sync.dma_start(gt[:], gb_d[j][:])
                gb_t.append(gt)

            inv_n = 1.0 / float(N)

            for j in range(6):
                fi, fo = DIMS[j]
                tbl = tbls[j]

                # per-stream gather/chunk state
                cur_chunk = [-1, -1]
                msg_tiles = [None, None]
                s_tiles = {}  # (w, h) -> S tile

                def ensure_chunk(h, t):
                    """Gather the chunk containing global tile t of stream h."""
                    k = 0
                    while not (
                        chunks[h][k][0] <= t < chunks[h][k][0] + chunks[h][k][1]
                    ):
                        k += 1
                    if cur_chunk[h] == k:
                        return k
                    cur_chunk[h] = k
                    t0c, ntc = chunks[h][k]
                    mt = msgp.tile([128, ntc, fi], FP32, tag="msg", name="msg")
                    nc.gpsimd.dma_gather(
                        out_ap=mt[:],
                        in_ap=tbl[h * HALF : (h + 1) * HALF, :],
                        idxs_ap=idx_t[h][
                            :, idx_col0[h][k] : idx_col0[h][k] + ntc * 8
                        ],
                        num_idxs=ntc * 128,
                        num_idxs_reg=ntc * 128,
                        elem_size=fi,
                        single_packet=False,
                    )
                    msg_tiles[h] = (k, t0c, mt)
                    return k

                y_sb = big.tile([128, SHP], FP32, tag="ysb", name="ysb")
                sumP = small.tile([128, NSB], FP32, tag="sumP", name="sumP")
                sqP = small.tile([128, NSB], FP32, tag="sqP", name="sqP")
                junk = small.tile([128, 512], FP32, tag="junk", name="junk")

                for sb in range(NSB):
                    nsb = 512 if sb < 12 else 128
                    wlist = list(range(sb * 4, min(sb * 4 + 4, NWIN)))
                    # collect (w, h, t) sequence for this superblock
                    seq = []
                    for w in wlist:
                        for h in (0, 1):
                            for t in range(t0s[w][h], t0s[w][h] + tiles[w][h]):
                                seq.append((w, h, t))
                    agg = aggp.tile([128, 512], FP32, tag="agg", name="agg")
                    for i, (w, h, t) in enumerate(seq):
                        ensure_chunk(h, t)
                        k, t0c, mt = msg_tiles[h]
                        if (w, h) not in s_tiles:
                            G = tiles[w][h]
                            st = sp.tile([128, G, 128], FP32, tag="S", name="S")
                            i0 = iota_t[:].rearrange(
                                "p (g d) -> p g d", g=1
                            ).broadcast_to([128, G, 128])
                            i1 = dstl_t[h][
                                :, t0s[w][h] : t0s[w][h] + G
                            ].broadcast_to([128, G, 128])
                            nc.vector.tensor_tensor(
                                st[:], i0, i1, op=OP.is_equal
                            )
                            s_tiles[(w, h)] = (st, t0s[w][h])
                        st, st_t0 = s_tiles[(w, h)]
                        woff = (w % 4) * 128
                        nc.tensor.matmul(
                            agg[0:fi, woff : woff + 128],
                            mt[:, t - t0c, 0:fi],
                            st[:, t - st_t0, :],
                            start=(i == 0),
                            stop=(i == len(seq) - 1),
                        )
                    # evict + dinv scale
                    dv = small.tile([128, 512], FP32, tag="dinv", name="dinv")
                    nc.sync.dma_start(
                        dv[:, 0:nsb], dinv_d[:, sb * 512 : sb * 512 + nsb]
                    )
                    rawT = small.tile([128, 512], FP32, tag="rawT", name="rawT")
                    nc.vector.tensor_tensor(
                        rawT[0:fi, 0:nsb],
                        agg[0:fi, 0:nsb],
                        dv[0:fi, 0:nsb],
                        op=OP.mult,
                    )
                    # W matmul
                    y_ps = yp.tile([128, 512], FP32, tag="yps", name="yps")
                    nc.tensor.matmul(
                        y_ps[0:fo, 0:nsb],
                        W_t[j][:],
                        rawT[0:fi, 0:nsb],
                        start=True,
                        stop=True,
                    )
                    # copy to y_sb + stats (valid columns only)
                    nv = 512 if sb < 12 else 106
                    c0 = sb * 512
                    nc.scalar.activation(
                        y_sb[0:fo, c0 : c0 + nv],
                        y_ps[0:fo, 0:nv],
                        ACTF.Copy,
                        accum_out=sumP[0:fo, sb : sb + 1],
                    )
                    if sb == 12:
                        nc.scalar.activation(
                            y_sb[0:fo, c0 + 106 : c0 + 128],
                            y_ps[0:fo, 106:128],
                            ACTF.Copy,
                        )
                    nc.scalar.activation(
                        junk[0:fo, 0:nv],
                        y_ps[0:fo, 0:nv],
                        ACTF.Square,
                        accum_out=sqP[0:fo, sb : sb + 1],
                    )

                # ---- BN stats all-reduce ----
                stats = small.tile([128, 2], FP32, tag="stats", name="stats")
                nc.vector.memset(stats[:], 0.0)
                nc.vector.reduce_sum(stats[0:fo, 0:1], sumP[0:fo, :], axis=AX)
                nc.vector.reduce_sum(stats[0:fo, 1:2], sqP[0:fo, :], axis=AX)
                nc.sync.dma_start(ar_in[j][:], stats[:])
                nc.gpsimd.collective_compute(
                    "AllReduce",
                    OP.add,
                    replica_groups=[list(range(NC))],
                    ins=[ar_in[j][:]],
                    outs=[ar_out[j][:]],
                )
                arr = small.tile([128, 2], FP32, tag="arr", name="arr")
                nc.sync.dma_start(arr[:], ar_out[j][:])

                # mean/var -> scale/shift
                vec = small.tile([128, 6], FP32, tag="bnvec", name="bnvec")
                # vec cols: 0 mean, 1 ex2, 2 var(+eps), 3 rstd, 4 scale, 5 shift
                nc.vector.tensor_scalar(
                    out=vec[0:fo, 0:1], in0=arr[0:fo, 0:1],
                    scalar1=inv_n, scalar2=None, op0=OP.mult,
                )
                nc.vector.tensor_scalar(
                    out=vec[0:fo, 1:2], in0=arr[0:fo, 1:2],
                    scalar1=inv_n, scalar2=None, op0=OP.mult,
                )
                nc.vector.tensor_tensor(
                    vec[0:fo, 2:3], vec[0:fo, 0:1], vec[0:fo, 0:1], op=OP.mult
                )
                nc.vector.tensor_tensor(
                    vec[0:fo, 2:3], vec[0:fo, 1:2], vec[0:fo, 2:3], op=OP.subtract
                )
                nc.vector.tensor_scalar(
                    out=vec[0:fo, 2:3], in0=vec[0:fo, 2:3],
                    scalar1=float(EPS), scalar2=None, op0=OP.add,
                )
                nc.vector.reciprocal(vec[0:fo, 3:4], vec[0:fo, 2:3])
                nc.scalar.activation(vec[0:fo, 3:4], vec[0:fo, 3:4], ACTF.Sqrt)
                nc.vector.tensor_tensor(
                    vec[0:fo, 4:5], gb_t[j][0:fo, 0:1], vec[0:fo, 3:4], op=OP.mult
                )
                nc.vector.tensor_tensor(
                    vec[0:fo, 5:6], vec[0:fo, 0:1], vec[0:fo, 4:5], op=OP.mult
                )
                nc.vector.tensor_tensor(
                    vec[0:fo, 5:6], gb_t[j][0:fo, 1:2], vec[0:fo, 5:6],
                    op=OP.subtract,
                )

                # ---- BN apply (+ReLU) in place ----
                fn = ACTF.Relu if RELU[j] else ACTF.Identity
                for sb in range(NSB):
                    nsb = 512 if sb < 12 else 128
                    c0 = sb * 512
                    nc.scalar.activation(
                        y_sb[0:fo, c0 : c0 + nsb],
                        y_sb[0:fo, c0 : c0 + nsb],
                        fn,
                        bias=vec[0:fo, 5:6],
                        scale=vec[0:fo, 4:5],
                    )

                if j == 5:
                    nc.sync.dma_start(out_d[:], y_sb[:])
                    continue

                # ---- rescale by dinv, transpose to node-major, AllGather ----
                for sb in range(NSB):
                    nsb = 512 if sb < 12 else 128
                    c0 = sb * 512
                    dv2 = small.tile([128, 512], FP32, tag="dinv", name="dinv")
                    nc.sync.dma_start(
                        dv2[:, 0:nsb], dinv_d[:, c0 : c0 + nsb]
                    )
                    nc.vector.tensor_tensor(
                        y_sb[0:fo, c0 : c0 + nsb],
                        y_sb[0:fo, c0 : c0 + nsb],
                        dv2[0:fo, 0:nsb],
                        op=OP.mult,
                    )
                hnext = big.tile([128, NWIN, fo], FP32, tag="hnext", name="hnext")
                for b in range(NWIN):
                    trt = trp.tile([128, 128], FP32, tag="tr", name="tr")
                    nc.tensor.transpose(
                        trt[0:128, 0:fo],
                        y_sb[0:fo, b * 128 : (b + 1) * 128],
                        ident_t[0:fo, 0:fo],
                    )
                    nc.vector.tensor_copy(hnext[:, b, :], trt[0:128, 0:fo])
                nc.sync.dma_start(
                    ag_in[j][:].rearrange("(b p) f -> p b f", p=128), hnext[:]
                )
                nc.gpsimd.collective_compute(
                    "AllGather",
                    OP.bypass,
                    replica_groups=[list(range(NC))],
                    ins=[ag_in[j][:]],
                    outs=[ag_out[j][:]],
                )

    nc.compile()
    return nc


def kernel(x, edge_index, **params):
    global LAST_RESULT
    from concourse.bass_utils import run_bass_kernel_spmd

    x = np.asarray(x, np.float32)
    edge_index = np.asarray(edge_index, np.int64)
    src_all = edge_index[0]
    dst_all = edge_index[1]

    deg = (np.bincount(dst_all, minlength=N) + 1.0).astype(np.float32)
    dinv = (1.0 / np.sqrt(deg)).astype(np.float32)

    hs0 = np.zeros((NP, F_IN), np.float32)
    xs = x * dinv[:, None]
    for c in range(NC):
        hs0[c * SHP : c * SHP + SH] = xs[c * SH : (c + 1) * SH]

    remap = (src_all // SH) * SHP + (src_all % SH)

    # per-core edge lists
    eds = []
    for c in range(NC):
        m = (dst_all >= c * SH) & (dst_all < (c + 1) * SH)
        dstl = dst_all[m] - c * SH
        srcr = remap[m]
        self_dstl = np.arange(SH, dtype=np.int64)
        self_src = c * SHP + self_dstl
        pad_dstl = np.arange(SH, SHP, dtype=np.int64)
        pad_src = c * SHP + pad_dstl  # rows that are zero in every table
        dstl = np.concatenate([dstl, self_dstl, pad_dstl])
        srcr = np.concatenate([srcr, self_src, pad_src])
        eds.append(_prep_edges(srcr, dstl))

    # uniform tile counts
    tiles = [[0, 0] for _ in range(NWIN)]
    for w in range(NWIN):
        for h in (0, 1):
            mx = max(len(eds[c][w][h][0]) for c in range(NC))
            tiles[w][h] = max(1, -(-mx // 128))
    t0s = [[0, 0] for _ in range(NWIN)]
    ta = tb = 0
    for w in range(NWIN):
        t0s[w][0] = ta
        ta += tiles[w][0]
        t0s[w][1] = tb
        tb += tiles[w][1]
    TA, TB = ta, tb

    chunksA = _chunks_of(TA)
    chunksB = _chunks_of(TB)

    in_maps = []
    for c in range(NC):
        (gA, dA), (gB, dB) = _build_core_tables(eds[c], tiles)
        dinvT = np.zeros(SHP, np.float32)
        dinvT[:SH] = dinv[c * SH : (c + 1) * SH]
        im = {
            "hs0": hs0,
            "idxA": _wrap_idx(gA, chunksA),
            "idxB": _wrap_idx(gB, chunksB),
            "dstlA": dA.reshape(TA, 128).T.copy(),
            "dstlB": dB.reshape(TB, 128).T.copy(),
            "dinvT": np.broadcast_to(dinvT, (128, SHP)).copy(),
            "iota": np.broadcast_to(
                np.arange(128, dtype=np.float32), (128, 128)
            ).copy(),
            "ident": np.eye(128, dtype=np.float32),
        }
        for j in range(6):
            im[f"W{j}"] = np.asarray(params[f"W{j}"], np.float32)
            gb = np.zeros((128, 2), np.float32)
            fo = DIMS[j][1]
            gb[:fo, 0] = np.asarray(params[f"g{j}"], np.float32)
            gb[:fo, 1] = np.asarray(params[f"be{j}"], np.float32)
            im[f"gb{j}"] = gb
        in_maps.append(im)

    nc = _build_program(tiles, t0s, TA, TB)
    res = run_bass_kernel_spmd(
        nc,
        in_maps,
        core_ids=list(range(NC)),
        trace=TRACE,
        **TRACE_KW,
    )
    LAST_RESULT = res

    out = np.empty((N, F_IN), np.float32)
    for c in range(NC):
        out[c * SH : (c + 1) * SH] = res.results[c]["out"].T[:SH]
    return out
